# revision 4
# baseline (speedup 1.0000x reference)
"""3x3 SAME conv + ReLU on 8 TRN2 cores — 1-D Winograd F(4,3) along H.

Each core gets 28 output rows (1-row halo). Per 4-row tile ty, the six
Winograd signals d_u = (B^T d)_u are built on DVE/Pool from just two
full-width helper arrays P[c] = x[c+2] - 4x[c] and S[c] = x[c+2] - x[c]
(w stays contiguous, so all transform ops run in DVE 2x/4x packed modes).
The GEMMs are weight-stationary with U[u,kw] = G-transformed weights;
the 3 kw taps accumulate into PSUM via shifted moving views. This needs
56,448 PE columns/core vs 115,200 for direct conv (2.04x fewer).

m_u land in PSUM pair-tiles (2 banks), ACT drains each pair with one
2-bank copy to fp16 SBUF, and the inverse transform
  y0 = m0+s1+s2, y1 = d1+2d2, y2 = s1+4s2, y3 = d1+8d2+m5
(s/d = sums/differences of m1..m4) runs on Pool + DVE in fp16, with a
4x-mode in-place ReLU. Output is stored oc-major [256, 6272] per core;
the host transposes while unsharding (DMA transpose XBAR only exists
for writes INTO SBUF).

All SBUF data is fp16 (PSUM fp32), keeping rel err ~3e-4 — far inside
the 2e-2 gate. Timing: no NTFF profiler exists under axon here, so
bench.py wraps this exact body in tc.For_i and wall-differences two
iteration counts on hardware.
"""

import sys
from contextlib import ExitStack, nullcontext

sys.path.insert(0, "/opt/trn_rl_repo")

import numpy as np

H = 224
WID = 224
C_IN = 128
C_OUT = 256
NCORES = 8
RPC = H // NCORES          # 28 output rows per core
XROWS = RPC + 2            # 30 input rows incl halo
WP = WID + 2               # 226 padded width
NPIX = XROWS * WP          # 6780
NPIXP = 6784               # padded to /16 for transpose DMA
TY = RPC // 4              # 7 winograd tiles of 4 rows
OPIX = RPC * WID           # 6272 output pixels per core
ROUNDS = [(0, 2), (2, 2), (4, 2), (6, 1)]  # (ty0, nty)
N_WARM = 26

_COMPILED = None
LAST_RESULT = None

# F(4,3) Winograd matrices (Lavin, correlation form)
G_MAT = np.array(
    [
        [1 / 4, 0, 0],
        [-1 / 6, -1 / 6, -1 / 6],
        [-1 / 6, 1 / 6, -1 / 6],
        [1 / 24, 1 / 12, 1 / 6],
        [1 / 24, -1 / 12, 1 / 6],
        [0, 0, 1],
    ],
    np.float64,
)
BT_MAT = np.array(
    [
        [4, 0, -5, 0, 1, 0],
        [0, -4, -4, 1, 1, 0],
        [0, 4, -4, -1, 1, 0],
        [0, -2, -1, 2, 1, 0],
        [0, 2, -1, -2, 1, 0],
        [0, 4, 0, -5, 0, 1],
    ],
    np.float64,
)
AT_MAT = np.array(
    [
        [1, 1, 1, 1, 1, 0],
        [0, 1, -1, 2, -2, 0],
        [0, 1, 1, 4, 4, 0],
        [0, 1, -1, 8, -8, 1],
    ],
    np.float64,
)


def _self_check():
    rng = np.random.default_rng(0)
    d = rng.standard_normal(6)
    g = rng.standard_normal(3)
    y = AT_MAT @ ((G_MAT @ g) * (BT_MAT @ d))
    y_ref = np.array([sum(g[k] * d[a + k] for k in range(3)) for a in range(4)])
    assert np.allclose(y, y_ref), (y, y_ref)


_self_check()


def build(iters=None):
    """Emit the kernel. iters=None -> single shot; else wrap body in For_i."""
    import concourse.bacc as bacc
    import concourse.mybir as mybir
    import concourse.tile as tile

    F16 = mybir.dt.float16
    F32 = mybir.dt.float32
    ADD = mybir.AluOpType.add
    SUB = mybir.AluOpType.subtract

    nc = bacc.Bacc("TRN2", target_bir_lowering=False, debug=False, num_devices=NCORES)

    xs_d = nc.dram_tensor("xs", [NPIXP, C_IN], F16, kind="ExternalInput").ap()
    w_d = nc.dram_tensor("w", [C_IN, 36 * 128], F16, kind="ExternalInput").ap()
    y_d = nc.dram_tensor("y", [C_OUT, OPIX], F16, kind="ExternalOutput").ap()

    with tile.TileContext(nc) as tc:
        with ExitStack() as stack:
            wp = stack.enter_context(tc.sbuf_pool(name="wp", bufs=1))
            xp = stack.enter_context(tc.sbuf_pool(name="xp", bufs=1))
            tp = stack.enter_context(tc.sbuf_pool(name="tp", bufs=2))
            cp = stack.enter_context(tc.sbuf_pool(name="cp", bufs=2))
            yp = stack.enter_context(tc.sbuf_pool(name="yp", bufs=3))
            pp = stack.enter_context(tc.psum_pool(name="pp", bufs=4))

            wt = wp.tile([C_IN, 36 * 128], F16)
            nc.sync.dma_start(wt, w_d)
            xt = xp.tile([C_IN, NPIXP], F16)

            # PE clock warmup on the weight tile (scratch psum pair).
            mwarm = pp.tile([128, 1024], F32, name="mp", tag="mp")
            for i in range(N_WARM):
                nc.tensor.matmul(
                    mwarm[:, 0:256], wt[:, 0:128], wt[:, 0:256],
                    start=(i == 0), stop=(i == N_WARM - 1),
                    skip_group_check=True,
                )

            loop = tc.For_i(0, iters) if iters is not None else nullcontext()
            with loop:
                nc.sync.dma_start(xt, xs_d, transpose=True)
                xr = xt[:, 0:NPIX].rearrange("p (r w) -> p r w", r=XROWS)

                def transform(r):
                    ty0, nty = ROUNDS[r]
                    c0, cn = 4 * ty0, 4 * nty
                    fw = cn * WP   # P free size
                    fs = nty * WP  # per-phase free size
                    # P[c] = x[c+2] - 4 x[c], c in [c0, c0+cn)
                    t4 = tp.tile([128, 8 * WP], F16, name="t4", tag="t4")
                    P = tp.tile([128, 8 * WP], F16, name="P", tag="P")
                    nc.vector.tensor_scalar_mul(
                        t4[:, 0:fw], xr[:, c0:c0 + cn, :], 4.0)
                    nc.vector.tensor_tensor(
                        P[:, 0:fw], xr[:, c0 + 2:c0 + cn + 2, :], t4[:, 0:fw], SUB)
                    Pv = P.rearrange("p (t w) -> p t w", w=WP)

                    def xph(d):  # x rows 4ty+d over round tiles
                        return xr[:, c0 + d:c0 + d + 4 * (nty - 1) + 1:4, :]

                    def pph(d):  # P rows 4ty+d (local)
                        return Pv[:, d:d + 4 * (nty - 1) + 1:4, :]

                    sp1 = tp.tile([128, 2 * WP], F16, name="sp1", tag="sp1")
                    sp2 = tp.tile([128, 2 * WP], F16, name="sp2", tag="sp2")
                    sx2 = tp.tile([128, 2 * WP], F16, name="sx2", tag="sx2")
                    nc.vector.tensor_tensor(sp1[:, 0:fs], xph(3), xph(1), SUB)
                    nc.vector.tensor_tensor(sp2[:, 0:fs], xph(4), xph(2), SUB)
                    nc.vector.tensor_scalar_mul(sx2[:, 0:fs], sp1[:, 0:fs], 2.0)

                    d_t = [
                        tp.tile([128, 2 * WP], F16, name=f"du{u}", tag=f"du{u}")
                        for u in range(6)
                    ]
                    nc.gpsimd.tensor_tensor(d_t[0][:, 0:fs], pph(2), pph(0), SUB)
                    nc.vector.tensor_tensor(d_t[1][:, 0:fs], pph(1), pph(2), ADD)
                    nc.vector.tensor_tensor(d_t[2][:, 0:fs], pph(2), pph(1), SUB)
                    nc.gpsimd.tensor_tensor(
                        d_t[3][:, 0:fs], sx2[:, 0:fs], sp2[:, 0:fs], ADD)
                    nc.gpsimd.tensor_tensor(
                        d_t[4][:, 0:fs], sp2[:, 0:fs], sx2[:, 0:fs], SUB)
                    nc.gpsimd.tensor_tensor(d_t[5][:, 0:fs], pph(3), pph(1), SUB)
                    return d_t

                def half_round(r, half, d_t):
                    ty0, nty = ROUNDS[r]
                    fr = nty * WID
                    # GEMMs into psum pair tiles: (m1,m2), (m3,m4), (m0,m5)
                    pairs = [(1, 2), (3, 4), (0, 5)]
                    mp_t = {}
                    for pa, pb in pairs:
                        mp = pp.tile([128, 1024], F32, name="mp", tag="mp")
                        for sl, u in ((slice(0, fr), pa),
                                      (slice(512, 512 + fr), pb)):
                            dv = d_t[u].rearrange("p (t w) -> p t w", w=WP)
                            for kw in range(3):
                                k = (u * 3 + kw) * 2 + half
                                nc.tensor.matmul(
                                    mp[:, sl],
                                    wt[:, k * 128:(k + 1) * 128],
                                    dv[:, 0:nty, kw:kw + WID],
                                    start=(kw == 0), stop=(kw == 2),
                                )
                        mp_t[(pa, pb)] = mp

                    # ACT: drain each psum pair with one 2-bank strided copy
                    def drain(pair):
                        c = cp.tile([128, 2, 448], F16,
                                    name=f"c{pair[0]}{pair[1]}",
                                    tag=f"c{pair[0]}{pair[1]}")
                        mv = mp_t[pair].rearrange("p (b f) -> p b f", b=2)
                        nc.scalar.copy(c[:, :, 0:fr], mv[:, :, 0:fr])
                        return c

                    c12 = drain((1, 2))
                    c34 = drain((3, 4))
                    c05 = drain((0, 5))

                    def st(name):
                        return cp.tile([128, 448], F16, name=name, tag=name)

                    s1, d1 = st("s1"), st("d1")
                    s2, d2 = st("s2"), st("d2")
                    d2x2, s2x4, d2x8 = st("d2x2"), st("s2x4"), st("d2x8")
                    t0, t3 = st("t0"), st("t3")

                    f = slice(0, fr)
                    # Pool: s1, d1, t0, y0, d2x8(TS), t3, y3
                    nc.gpsimd.tensor_tensor(s1[:, f], c12[:, 0, f], c12[:, 1, f], ADD)
                    nc.gpsimd.tensor_tensor(d1[:, f], c12[:, 0, f], c12[:, 1, f], SUB)
                    # DVE: s2, d2, y1, y2
                    nc.vector.tensor_tensor(s2[:, f], c34[:, 0, f], c34[:, 1, f], ADD)
                    nc.vector.tensor_tensor(d2[:, f], c34[:, 0, f], c34[:, 1, f], SUB)

                    yt = yp.tile([128, 8 * WID], F16, name="yt", tag="yt")
                    yv = yt.rearrange("p (a w) -> p a w", w=WID)

                    def yrow(a):  # rows 4*(ty-ty0)+a within this round
                        return yv[:, a:4 * nty:4, :]

                    nc.gpsimd.tensor_tensor(t0[:, f], c05[:, 0, f], s1[:, f], ADD)
                    nc.gpsimd.tensor_tensor(yrow(0), t0[:, f], s2[:, f], ADD)
                    nc.vector.tensor_scalar_mul(d2x2[:, f], d2[:, f], 2.0)
                    nc.vector.tensor_tensor(yrow(1), d1[:, f], d2x2[:, f], ADD)
                    nc.vector.tensor_scalar_mul(s2x4[:, f], s2[:, f], 4.0)
                    nc.vector.tensor_tensor(yrow(2), s1[:, f], s2x4[:, f], ADD)
                    nc.gpsimd.tensor_scalar_mul(d2x8[:, f], d2[:, f], 8.0)
                    nc.gpsimd.tensor_tensor(t3[:, f], d1[:, f], d2x8[:, f], ADD)
                    nc.gpsimd.tensor_tensor(yrow(3), t3[:, f], c05[:, 1, f], ADD)

                    # ReLU in place (packed 4x mode), then store
                    fy = slice(0, 4 * nty * WID)
                    nc.vector.tensor_scalar_max(yt[:, fy], yt[:, fy], 0.0)
                    nc.sync.dma_start(
                        y_d[half * 128:(half + 1) * 128,
                            4 * ty0 * WID:(4 * ty0 + 4 * nty) * WID],
                        yt[:, fy],
                    )

                d_prev = transform(0)
                for r in range(len(ROUNDS)):
                    d_cur = d_prev
                    half_round(r, 0, d_cur)
                    if r + 1 < len(ROUNDS):
                        d_prev = transform(r + 1)
                    half_round(r, 1, d_cur)

    nc.compile()
    return nc


def _prep_inputs(x: np.ndarray, W: np.ndarray):
    xpad = np.zeros((H + 2, WP, C_IN), np.float32)
    xpad[1:H + 1, 1:WID + 1] = x
    xs = np.zeros((NCORES, NPIXP, C_IN), np.float16)
    for i in range(NCORES):
        xs[i, 0:NPIX] = xpad[RPC * i:RPC * i + XROWS].reshape(NPIX, C_IN)

    Wt = W.reshape(C_OUT, 3, 3, C_IN).transpose(1, 2, 3, 0)  # [kh,kw,cin,oc]
    U = np.einsum("uk,kwco->uwco", G_MAT, Wt.astype(np.float64))  # [6,3,cin,oc]
    wh = np.empty((C_IN, 36 * 128), np.float16)
    for u in range(6):
        for kw in range(3):
            for half in range(2):
                k = (u * 3 + kw) * 2 + half
                wh[:, k * 128:(k + 1) * 128] = U[u, kw, :, half * 128:(half + 1) * 128]
    return xs, wh


def kernel(x: np.ndarray, W: np.ndarray) -> np.ndarray:
    global _COMPILED, LAST_RESULT
    from concourse import bass_utils

    if _COMPILED is None:
        _COMPILED = build()
    nc = _COMPILED

    xs, wh = _prep_inputs(np.asarray(x, np.float32), np.asarray(W, np.float32))
    in_maps = [{"xs": np.ascontiguousarray(xs[i]), "w": wh} for i in range(NCORES)]

    import os
    os.environ.pop("BASS_TRACE", None)
    res = bass_utils.run_bass_kernel_spmd(nc, in_maps, core_ids=list(range(NCORES)))
    LAST_RESULT = res

    y = np.stack([r["y"] for r in res.results])  # [8, 256, 6272] fp16
    y = y.reshape(NCORES, C_OUT, RPC, WID).transpose(0, 2, 3, 1)
    return y.reshape(H, WID, C_OUT).astype(np.float32)
